# revision 2
# baseline (speedup 1.0000x reference)
"""Causal linear attention (elu+1 feature map) for Trainium2, 8 NeuronCores.

Problem: B=2, S=2048, D=1024, H=16, HD=64.
  q/k/v projections [S,D]@[D,H*HD], phi = elu+1, causal linear attention
  out[t] = (sum_{i<=t} (phi_q[t].phi_k[i]) v[i]) / (phi_q[t].sum_{i<=t} phi_k[i] + eps)

Sharding: core c -> (batch b=c//4, heads h0=4*(c%4) .. h0+3). No cross-core comm.

v2: fp8 DoubleRow projections + fp8 wire format + host-side normalize.
  - Wire: xq8 = fp8(xq/16), xk8 = fp8(xk/16), wq8/wk8 = fp8(16*w) so the q/k
    psum is at true logit scale (1 Act + 2 DVE phi, no descale pass).
    xv8 = fp8(xv), xvr = fp8(xv - xv8), wv8 = fp8(16*wv), wvr = residual;
    v psum = 16*v via 3 DoubleRow terms (x8*w8 + x8*wr + xr*w8), descale 1/16
    folded into the v_aug evacuation. Accuracy (vs fp32 ref, same seed): 6.4e-3.
  - Projections on PE at 0.5 cycles/row (DoubleRow, 256-deep contraction):
    q/k 8.2k cycles each, v 24.6k, vs 98k for bf16 - PE drops ~31us.
  - DMA ~10MB/core (was 14.5MB bf16): x 8MB fp8 + w 1MB + out 1MB bf16.
  - phi(z) = max(z,0) + min(exp(z),1) (== elu(z)+1): t2 = Act Exp(psum),
    t2c = DVE min(t2, 1) [bf16 2x], phi = DVE stt(psum, 0, max, +t2c).
  - Attention phase keeps the v1 structure (per-parity A banks [A|T],
    head-pair-packed state, ones-column normalizer) but:
      * ONE mask op per chunk (both parity banks in one strided DVE op),
        phi_ks evacuated with a permuting DVE copy so the state matmul
        weights AP stays single-free-dim (walrus requirement),
      * op psum num|den evacuated raw (pair-merged, Act), host divides,
      * v_aug evac pair-merged on Act (v chunk-pairs share one psum bank
        via sequential groups), S snapshot on Act.
  - PSUM exactly 8 banks: proj 2, v 1, A 2, op 2, state 1.
  - Engine budgets (cost model): PE ~31us, DMA ~29, DVE ~30, Act ~27.
  NOTE: is_transpose=True matmuls with bf16 psum output pass the cost model
  and walrus but fail at runtime on hardware (INTERNAL error) - do not use.
"""

import os
import threading

import numpy as np

B, S, D, H, HD = 2, 2048, 1024, 16, 64
N_CORES = 8
HPC = 4            # heads per core
HDC = HPC * HD     # 256 projected cols per core
NCHUNK = S // 128  # 16
DC = D // 128      # 8 contraction chunks (4 DoubleRow pairs)
NQ = 4             # S quarters
CPQ = NCHUNK // NQ  # chunks per quarter
QS = S // NQ       # 512

_lock = threading.Lock()
_cache = {}


def _build_nc():
    import concourse.bass as bass
    import concourse.tile as tile
    from concourse import bacc, mybir

    f32 = mybir.dt.float32
    bf16 = mybir.dt.bfloat16
    fp8 = mybir.dt.float8e4
    Alu = mybir.AluOpType
    Act = mybir.ActivationFunctionType
    DR = mybir.MatmulPerfMode.DoubleRow

    nc = bacc.Bacc("TRN2", target_bir_lowering=False, debug=False)

    xq8 = nc.dram_tensor("xq8", [128, DC, S], fp8, kind="ExternalInput").ap()
    xk8 = nc.dram_tensor("xk8", [128, DC, S], fp8, kind="ExternalInput").ap()
    xv8 = nc.dram_tensor("xv8", [128, DC, S], fp8, kind="ExternalInput").ap()
    xvr = nc.dram_tensor("xvr", [128, DC, S], fp8, kind="ExternalInput").ap()
    wq8 = nc.dram_tensor("wq8", [128, DC, HDC], fp8, kind="ExternalInput").ap()
    wk8 = nc.dram_tensor("wk8", [128, DC, HDC], fp8, kind="ExternalInput").ap()
    wv8 = nc.dram_tensor("wv8", [128, DC, HDC], fp8, kind="ExternalInput").ap()
    wvr = nc.dram_tensor("wvr", [128, DC, HDC], fp8, kind="ExternalInput").ap()
    # per chunk: [t, (par, hp, 65)] with num 0:64, den col 64 per (par, hp)
    out = nc.dram_tensor("out", [NCHUNK, 128, 260], bf16, kind="ExternalOutput").ap()

    W0, W1, W2, W3 = (int(v) for v in os.environ.get(
        "KWARM", "8,6,8,10").split(","))
    F_OFF, B_OFF = (int(v) for v in os.environ.get("KSCHED", "6,8").split(","))

    with tile.TileContext(nc) as tc:
        with (
            tc.tile_pool(name="consts", bufs=1) as consts,
            tc.tile_pool(name="weights", bufs=1) as wpool,
            tc.tile_pool(name="resident", bufs=1) as res,
            tc.tile_pool(name="xin", bufs=16) as xin,
            tc.tile_pool(name="work", bufs=3) as work,
            tc.tile_pool(name="attn", bufs=2) as attn,
            tc.tile_pool(name="ps_proj", bufs=2, space="PSUM") as ps_proj,
            tc.tile_pool(name="ps_v", bufs=1, space="PSUM") as ps_v,
            tc.tile_pool(name="ps_a", bufs=1, space="PSUM") as ps_a,
            tc.tile_pool(name="ps_op", bufs=1, space="PSUM") as ps_op,
            tc.tile_pool(name="ps_state", bufs=1, space="PSUM") as ps_state,
        ):
            # ---- constants ----
            ones_bf = consts.tile([128, 128], bf16)
            nc.vector.memset(ones_bf[:], 1.0)
            ident = consts.tile([128, 128], bf16)
            nc.gpsimd.affine_select(
                ident[:], ones_bf[:], pattern=[[-1, 128]], base=0,
                channel_multiplier=1, compare_op=Alu.is_equal, fill=0.0,
            )
            onesw = consts.tile([128, 512], f32)
            nc.vector.memset(onesw[:], 1.0)
            onesb = consts.tile([128, 512], bf16)
            nc.vector.memset(onesb[:], 1.0)
            # causal mask in A-bank layout [j, (i, t)]: keep j <= t
            maskA = consts.tile([128, 256], bf16)
            nc.gpsimd.affine_select(
                maskA[:].rearrange("p (g t) -> p g t", g=2),
                onesb[:, 0:256].rearrange("p (g t) -> p g t", g=2),
                pattern=[[0, 2], [1, 128]], base=0,
                channel_multiplier=-1, compare_op=Alu.is_ge, fill=0.0,
            )

            # ---- weights ----
            w_sb = {}
            for name, wdram in (("q", wq8), ("k", wk8), ("v", wv8), ("vr", wvr)):
                w_sb[name] = wpool.tile([128, DC, HDC], fp8, name=f"w{name}_sb")

            xt = {}

            def load_quarter(xdram, qt, tag, split=1):
                t = xin.tile([128, DC, QS], fp8, name=f"x_{tag}_{qt}", tag="xin")
                src = xdram[:, :, qt * QS:(qt + 1) * QS]
                w = DC // split
                dmas = []
                for i in range(split):
                    sl = slice(w * i, w * i + w)
                    dmas.append(lambda sl=sl: nc.sync.dma_start(t[:, sl], src[:, sl]))
                return t, dmas

            # startup: wq, wk, then first quarters (dc-split for early starts)
            nc.sync.dma_start(w_sb["q"][:], wq8)
            nc.sync.dma_start(w_sb["k"][:], wk8)
            t, dmas = load_quarter(xq8, 0, "q", split=2)
            xt[("q", 0)] = t
            dmas[0]()
            nc.sync.dma_start(w_sb["v"][:], wv8)
            dmas[1]()
            t, dmas = load_quarter(xk8, 0, "k", split=2)
            xt[("k", 0)] = t
            dmas[0]()
            nc.sync.dma_start(w_sb["vr"][:], wvr)
            dmas[1]()
            for tag, dram in (("v", xv8), ("vr", xvr)):
                t, dmas = load_quarter(dram, 0, tag, split=2)
                xt[(tag, 0)] = t
                for d in dmas:
                    d()
            for qt in range(1, NQ):
                for tag, dram in (("q", xq8), ("k", xk8), ("v", xv8), ("vr", xvr)):
                    t, dmas = load_quarter(dram, qt, tag, split=2)
                    xt[(tag, qt)] = t
                    for d in dmas:
                        d()

            # ---- resident activations ----
            phi_qT = [[res.tile([128, QS], bf16, name=f"phi_qT{i}_{q}")
                       for q in range(NQ)] for i in range(2)]
            phi_kT = [[res.tile([128, QS], bf16, name=f"phi_kT{i}_{q}")
                       for q in range(NQ)] for i in range(2)]
            v_aug = res.tile([128, NCHUNK, HPC, 65], bf16, name="v_aug")
            nc.vector.memset(v_aug[:, :, :, 64:65], 1.0)

            # persistent KV state: head-pair blocks [(par, e), (hp, 65)]
            S_ps = ps_state.tile([128, 2 * 130], f32, name="S_ps")

            warm_budget = [True]

            def warm(n):
                if not warm_budget[0]:
                    return
                for _ in range(n):
                    nc.tensor.matmul(S_ps[:, 0:128], ones_bf[:], ones_bf[:],
                                     start=True, stop=True)

            # ---- projection units ----
            def qk_unit(qt, tname, dst, hp):
                def emit_mm():
                    x = xt[(tname, qt)]
                    ps = ps_proj.tile([128, QS], f32, tag="proj",
                                      name=f"ps_{tname}_{qt}_{hp}")
                    # two sequential 256-col groups in one bank (moving free
                    # limit 512 = 2 k-tiles x 256)
                    for nh in range(2):
                        ns = slice(nh * 256, (nh + 1) * 256)
                        for dcp in range(DC // 2):
                            nc.tensor.matmul(
                                ps[:, ns],
                                w_sb[tname][:, 2 * dcp:2 * dcp + 2,
                                            hp * 128:(hp + 1) * 128],
                                x[:, 2 * dcp:2 * dcp + 2, ns],
                                start=(dcp == 0), stop=(dcp == DC // 2 - 1),
                                perf_mode=DR,
                            )
                    return ps

                def emit_post(ps):
                    # phi(z) = max(z,0) + min(exp(z),1): exp on Act, min on
                    # DVE (2x bf16), fused relu+add from psum on DVE
                    t2 = work.tile([128, QS], bf16, tag="phit2")
                    nc.scalar.activation(t2[:], ps[:], Act.Exp)
                    t2c = work.tile([128, QS], bf16, tag="phit2c")
                    nc.vector.tensor_tensor(t2c[:], t2[:], onesb[:], op=Alu.min)
                    nc.vector.scalar_tensor_tensor(
                        dst[hp][qt][:], ps[:], 0.0, t2c[:],
                        op0=Alu.max, op1=Alu.add)
                return emit_mm, emit_post

            def v_unit(qt, cc):
                c = qt * CPQ + cc

                def emit_mm():
                    # chunk-pairs share one bank: even chunk cols 0:256 (group
                    # closes), odd chunk cols 256:512 (second group)
                    if c % 2 == 0:
                        _vst["ps"] = ps_v.tile([128, 512], f32, tag="vps",
                                               name=f"ps_v_{c // 2}")
                    ps = _vst["ps"]
                    off = (c % 2) * 256
                    terms = (("v", "v"), ("v", "vr"), ("vr", "v"))
                    n_mm = len(terms) * (DC // 2)
                    i = 0
                    for xtag, wtag in terms:
                        x = xt[(xtag, qt)]
                        for dcp in range(DC // 2):
                            nc.tensor.matmul(
                                ps[:, off:off + 256],
                                x[:, 2 * dcp:2 * dcp + 2,
                                  cc * 128:cc * 128 + 128],
                                w_sb[wtag][:, 2 * dcp:2 * dcp + 2, :],
                                start=(i == 0), stop=(i == n_mm - 1),
                                perf_mode=DR,
                            )
                            i += 1
                    return ps

                def emit_post(ps):
                    if c % 2 == 1:
                        # pair-merged evac with 1/16 descale (Act: scaled copy)
                        nc.scalar.activation(
                            v_aug[:, c - 1:c + 1, :, 0:64],
                            ps[:].rearrange("p (c h e) -> p c h e", c=2, h=HPC),
                            Act.Copy, scale=0.0625)
                return emit_mm, emit_post

            # ---- attention ----
            st = {"S_sb": None, "aop": {}, "ops": {}}

            def attn_front(c):
                """A + transposes per parity into [A(2x128) | T(2x64)] banks
                (one bank per parity, baseline layout)."""
                qc = c // CPQ
                cs = slice((c % CPQ) * 128, (c % CPQ + 1) * 128)
                a_ps = ps_a.tile([128, 2, 512], f32, tag="A", name=f"a_ps_{c}")
                for par in range(2):
                    hb = 64 * par
                    for i, hp in enumerate((0, 1)):
                        nc.tensor.matmul(
                            a_ps[:, par, i * 128:(i + 1) * 128],
                            phi_kT[hp][qc][hb:hb + 64, cs],
                            phi_qT[hp][qc][hb:hb + 64, cs],
                            start=(i == 0), stop=False,
                        )
                    for i, hp in enumerate((0, 1)):
                        nc.tensor.matmul(
                            a_ps[:, par, 256 + i * 64:256 + (i + 1) * 64],
                            phi_kT[hp][qc][hb:hb + 64, cs],
                            ident[hb:hb + 64, hb:hb + 64],
                            start=False, stop=(i == 1),
                        )
                # masked a_sb [j, par, (i, t)]
                ak = attn.tile([128, 2, 256], bf16, tag="ak", name=f"ak_{c}")
                nc.vector.tensor_tensor(
                    ak[:], a_ps[:, :, 0:256],
                    maskA[:].rearrange("p (o n) -> p o n", o=1).broadcast_to(
                        [128, 2, 256]),
                    op=Alu.mult)
                # phi_ks [t, hp, par, e]: permuting evac so the state matmul
                # weights AP is contiguous per hp
                pks = attn.tile([128, 2, 2, 64], bf16, tag="pks", name=f"pks_{c}")
                nc.vector.tensor_copy(
                    pks[:],
                    a_ps[:, :, 256:384].rearrange("p a (i e) -> p i a e", i=2))
                st["aop"][c] = (ak, pks)

            def attn_back(c):
                qc = c // CPQ
                cs = slice((c % CPQ) * 128, (c % CPQ + 1) * 128)
                S_sb = st["S_sb"]
                ak, pks = st["aop"].pop(c)
                # op: [128, 2, 512] = 2 banks, dim1 = chunk parity; parity
                # groups sequential within a bank; pair-merged evac
                if c % 2 == 0:
                    st["op"] = ps_op.tile([128, 2, 512], f32, tag="op",
                                          name=f"op_ps_{c // 2}")
                op = st["op"]
                cb = c % 2
                for par in range(2):
                    hb = 64 * par
                    n_mm = 2 + (0 if c == 0 else 2)
                    i_mm = 0
                    for i, hp in enumerate((0, 1)):
                        h = 2 * hp + par
                        nc.tensor.matmul(
                            op[:, cb, par * 130 + i * 65:par * 130 + (i + 1) * 65],
                            ak[:, par, i * 128:(i + 1) * 128],
                            v_aug[:, c, h, :],
                            start=(i_mm == 0), stop=(i_mm == n_mm - 1),
                        )
                        i_mm += 1
                    if c > 0:
                        for i, hp in enumerate((0, 1)):
                            nc.tensor.matmul(
                                op[:, cb, par * 130 + i * 65:par * 130 + (i + 1) * 65],
                                phi_qT[hp][qc][hb:hb + 64, cs],
                                S_sb[hb:hb + 64,
                                     130 * hp + 65 * par:130 * hp + 65 * par + 65],
                                start=False, stop=(i_mm == n_mm - 1),
                            )
                            i_mm += 1
                # state increment (one long persistent psum group)
                for hp in range(2):
                    nc.tensor.matmul(
                        S_ps[:, 130 * hp:130 * (hp + 1)],
                        pks[:, hp],
                        v_aug[:, c, 2 * hp:2 * hp + 2, :],
                        start=(c == 0 and hp == 0),
                        stop=(c == NCHUNK - 1 and hp == 1),
                    )
                if c < NCHUNK - 1:
                    S_new = attn.tile([128, 2 * 130], bf16, tag="Ssb",
                                      name=f"S_sb_{c}")
                    nc.scalar.activation(S_new[:], S_ps[:], Act.Copy)
                    st["S_sb"] = S_new
                # pair-merged raw num|den evac + store (host divides)
                if c % 2 == 1:
                    o_sb = work.tile([128, 2, 260], bf16, tag="osb",
                                     name=f"o_sb_{c // 2}", bufs=2)
                    nc.scalar.activation(o_sb[:], op[:, :, 0:260], Act.Copy)
                    nc.sync.dma_start(
                        out[c - 1:c + 1].rearrange("c p n -> p c n"), o_sb[:])

            # ---- schedule ----
            def units_of(qt):
                return [qk_unit(qt, "q", phi_qT, 0), qk_unit(qt, "q", phi_qT, 1),
                        qk_unit(qt, "k", phi_kT, 0), qk_unit(qt, "k", phi_kT, 1),
                        v_unit(qt, 0), v_unit(qt, 1), v_unit(qt, 2), v_unit(qt, 3)]

            _vst = {}
            slots = {}
            for c in range(NCHUNK):
                qc, cc = c // CPQ, c % CPQ
                slots.setdefault(8 * qc + F_OFF + cc, []).append(("F", c))
                slots.setdefault(8 * qc + B_OFF + cc, []).append(("B", c))
            units = [u for qt in range(NQ) for u in units_of(qt)]
            warm(W0)
            n_slots = max(32, max(slots) + 1)
            for slot in range(n_slots):
                post = None
                if slot < 32:
                    mm, post_fn = units[slot]
                    ps = mm()
                    post = (post_fn, ps)
                if slot == 3:
                    warm(W1)
                elif slot == 5:
                    warm(W2)
                elif slot == 7:
                    warm(W3)
                    warm_budget[0] = False
                for kind, c in sorted(slots.get(slot, [])):
                    (attn_back if kind == "B" else attn_front)(c)
                if post is not None:
                    post[0](post[1])

    nc.compile()
    return nc


def _get_nc():
    with _lock:
        if "nc" not in _cache:
            _cache["nc"] = _build_nc()
        return _cache["nc"]


def kernel(query, key, value, query_kernel, key_kernel, value_kernel):
    import ml_dtypes
    from concourse.bass_utils import run_bass_kernel_spmd

    nc = _get_nc()
    f8 = ml_dtypes.float8_e4m3

    def x_arrange(x):  # [S, D] fp32 -> [128, DC, S]
        return np.ascontiguousarray(x.T.reshape(DC, 128, S).transpose(1, 0, 2))

    xmaps = {}
    for b in range(B):
        xq = x_arrange(query[b] / 16.0).astype(f8)
        xk = x_arrange(key[b] / 16.0).astype(f8)
        xv_f = x_arrange(value[b])
        xv8 = xv_f.astype(f8)
        xvr = (xv_f - xv8.astype(np.float32)).astype(f8)
        xmaps[b] = (xq, xk, xv8, xvr)

    def w_arrange(wk_full, h0, scale=16.0):  # -> [128, DC, HDC] fp32
        w = (wk_full[:, h0:h0 + HPC, :] * scale).reshape(D, HDC)
        return np.ascontiguousarray(
            w.reshape(DC, 128, HDC).transpose(1, 0, 2))

    in_maps = []
    for c in range(N_CORES):
        b, h0 = c // 4, 4 * (c % 4)
        wq = w_arrange(query_kernel, h0).astype(f8)
        wk = w_arrange(key_kernel, h0).astype(f8)
        wv_f = w_arrange(value_kernel, h0)
        wv8 = wv_f.astype(f8)
        wvr = (wv_f - wv8.astype(np.float32)).astype(f8)
        xq, xk, xv8, xvr = xmaps[b]
        in_maps.append({
            "xq8": xq, "xk8": xk, "xv8": xv8, "xvr": xvr,
            "wq8": wq, "wk8": wk, "wv8": wv8, "wvr": wvr,
        })

    results = run_bass_kernel_spmd(nc, in_maps, core_ids=list(range(N_CORES)))

    # out[c, t, par*130 + hp*65 + (0:64|64)] = num|den of head 2hp+par.
    # Reference ends with a flat (B*H, S, HD) -> (B, S, H*HD) reshape:
    # output rows 128h..128h+128 of batch b are head h's [S, HD] flat.
    full = np.empty((B, S, H * HD), dtype=np.float32)
    for c in range(N_CORES):
        b, h0 = c // 4, 4 * (c % 4)
        o = np.asarray(results.results[c]["out"]).astype(np.float32)
        o = o.reshape(S, 260)
        for hl in range(HPC):
            hp, par = hl // 2, hl % 2
            base = par * 130 + hp * 65
            num = o[:, base:base + 64]
            den = o[:, base + 64] + 1e-6
            av = num / den[:, None]                      # [S, HD]
            full[b, (h0 + hl) * 128:(h0 + hl + 1) * 128, :] = (
                av.reshape(128, H * HD))
    return full
